# revision 3
# baseline (speedup 1.0000x reference)
"""Trainium2 Bass kernel for AutoLabelClusterCrossEntropyLoss (vq_codebook).

Problem: embeddings [262144, 128], centroids [1024, 128].
  dist[n,k] = ||e_n - c_k||;  labels = argmin_k dist;
  loss = mean_n(logsumexp_k(-dist) + min_k dist)

Data-parallel over 8 NeuronCores, 32768 rows each. Per 128-row tile
(rows on partitions, centroids on the free dim):
  TensorE : q = -2*e@c.T + c2   -- bf16 hi/lo split matmuls (hi*hi+hi*lo+lo*hi)
            plus a ones-row matmul adding the 3-way bf16 split of c2.
  ScalarE : dist = sqrt(q + e2[row])     (activation, e2 as per-partition bias)
  ScalarE : y = exp(-dist), accum_out -> ssum[row] = sum_k y
  VectorE : max8(y), max_index(y)  -> top value exp(-min dist) and argmin label
Host: labels = idx[:,0]; loss = mean(log(ssum) - log(ymax)) ... note
  min dist m = -log(ymax), computed on host in f64.
sqrt and exp live in different ACT table sets, so tiles are processed in
groups of G: all sqrts of a group, then all exps, amortizing the ~2.7us
table switch.
"""

import numpy as np

N, D, K = 262144, 128, 1024
NCORES = 8
NS = N // NCORES            # rows per core
P = 128                     # partitions / tile rows
T = NS // P                 # 256 tiles per core
G = 16                      # tiles per ACT table-set group
NGROUPS = T // G

_compiled = None


def _build():
    import concourse.bass as bass
    import concourse.tile as tile
    from concourse import bacc, mybir

    f32 = mybir.dt.float32
    bf16 = mybir.dt.bfloat16
    u32 = mybir.dt.uint32
    Act = mybir.ActivationFunctionType

    nc = bacc.Bacc("TRN2", target_bir_lowering=False, debug=False,
                   num_devices=NCORES)

    ethi_d = nc.dram_tensor("ethi", [P, NS], bf16, kind="ExternalInput")
    etlo_d = nc.dram_tensor("etlo", [P, NS], bf16, kind="ExternalInput")
    cmhi_d = nc.dram_tensor("cmhi", [P, K], bf16, kind="ExternalInput")
    cmlo_d = nc.dram_tensor("cmlo", [P, K], bf16, kind="ExternalInput")
    ones_d = nc.dram_tensor("ones3", [3, P], bf16, kind="ExternalInput")
    c23_d = nc.dram_tensor("c23", [3, K], bf16, kind="ExternalInput")
    e2_d = nc.dram_tensor("e2t", [P, T], f32, kind="ExternalInput")
    ym_d = nc.dram_tensor("ymax8", [P, 8 * T], f32, kind="ExternalOutput")
    ix_d = nc.dram_tensor("idx8", [P, 8 * T], u32, kind="ExternalOutput")
    ss_d = nc.dram_tensor("ssum", [P, T], f32, kind="ExternalOutput")

    with tile.TileContext(nc) as tc:
        with (
            tc.tile_pool(name="const", bufs=1) as constp,
            tc.tile_pool(name="etg", bufs=3) as etp,
            tc.tile_pool(name="psum", bufs=4, space="PSUM") as psump,
            tc.tile_pool(name="dist", bufs=G + 2) as distp,
            tc.tile_pool(name="y", bufs=10) as yp,
            tc.tile_pool(name="acc", bufs=1) as accp,
        ):
            cmhi_sb = constp.tile([P, K], bf16, tag="cmhi")
            nc.sync.dma_start(cmhi_sb[:], cmhi_d.ap())
            cmlo_sb = constp.tile([P, K], bf16, tag="cmlo")
            nc.sync.dma_start(cmlo_sb[:], cmlo_d.ap())
            ones_sb = constp.tile([3, P], bf16, tag="ones")
            nc.sync.dma_start(ones_sb[:], ones_d.ap())
            c23_sb = constp.tile([3, K], bf16, tag="c23")
            nc.sync.dma_start(c23_sb[:], c23_d.ap())
            e2_sb = constp.tile([P, T], f32, tag="e2")
            nc.sync.dma_start(e2_sb[:], e2_d.ap())

            ym_sb = accp.tile([P, 8 * T], f32, tag="ym")
            ix_sb = accp.tile([P, 8 * T], u32, tag="ix")
            ss_sb = accp.tile([P, T], f32, tag="ss")

            for g in range(NGROUPS):
                ethi_g = etp.tile([P, G * P], bf16, tag="ethi")
                nc.sync.dma_start(ethi_g[:], ethi_d.ap()[:, g * G * P:(g + 1) * G * P])
                etlo_g = etp.tile([P, G * P], bf16, tag="etlo")
                nc.sync.dma_start(etlo_g[:], etlo_d.ap()[:, g * G * P:(g + 1) * G * P])

                dists = []
                for tl in range(G):
                    t = g * G + tl
                    ehi = ethi_g[:, tl * P:(tl + 1) * P]
                    elo = etlo_g[:, tl * P:(tl + 1) * P]
                    q = psump.tile([P, K], f32, tag="q")
                    for h in (0, 1):
                        sl = slice(h * 512, (h + 1) * 512)
                        nc.tensor.matmul(q[:, sl], ehi, cmhi_sb[:, sl],
                                         start=True, stop=False)
                        nc.tensor.matmul(q[:, sl], ehi, cmlo_sb[:, sl],
                                         start=False, stop=False)
                        nc.tensor.matmul(q[:, sl], elo, cmhi_sb[:, sl],
                                         start=False, stop=False)
                        nc.tensor.matmul(q[:, sl], ones_sb[:], c23_sb[:, sl],
                                         start=False, stop=True)

                    dist = distp.tile([P, K], f32, tag="dist")
                    nc.scalar.activation(
                        out=dist[:], in_=q[:], func=Act.Sqrt,
                        bias=e2_sb[:, t:t + 1], scale=1.0,
                    )
                    dists.append(dist)

                for tl in range(G):
                    t = g * G + tl
                    y = yp.tile([P, K], f32, tag="y")
                    nc.scalar.activation(
                        out=y[:], in_=dists[tl][:], func=Act.Exp,
                        bias=0.0, scale=-1.0,
                        accum_out=ss_sb[:, t:t + 1],
                    )
                    ym8 = ym_sb[:, t * 8:(t + 1) * 8]
                    nc.vector.max(ym8, y[:])
                    nc.vector.max_index(ix_sb[:, t * 8:(t + 1) * 8], ym8, y[:])

            nc.sync.dma_start(ym_d.ap(), ym_sb[:])
            nc.sync.dma_start(ix_d.ap(), ix_sb[:])
            nc.sync.dma_start(ss_d.ap(), ss_sb[:])

    nc.compile()
    return nc


def _get_compiled():
    global _compiled
    if _compiled is None:
        _compiled = _build()
    return _compiled


def _bf16_split(x):
    import ml_dtypes
    hi = x.astype(ml_dtypes.bfloat16)
    lo = (x - hi.astype(np.float32)).astype(ml_dtypes.bfloat16)
    return np.ascontiguousarray(hi), np.ascontiguousarray(lo)


def _bf16_split3(x):
    import ml_dtypes
    h = x.astype(ml_dtypes.bfloat16)
    r = x - h.astype(np.float32)
    m = r.astype(ml_dtypes.bfloat16)
    l = (r - m.astype(np.float32)).astype(ml_dtypes.bfloat16)
    return h, m, l


def kernel(embeddings: np.ndarray, cluster_centroids: np.ndarray):
    import ml_dtypes
    from concourse.bass_utils import run_bass_kernel_spmd

    e = np.ascontiguousarray(np.asarray(embeddings, dtype=np.float32))
    c = np.ascontiguousarray(np.asarray(cluster_centroids, dtype=np.float32))
    assert e.shape == (N, D) and c.shape == (K, D)

    nc = _get_compiled()

    cm = (-2.0 * c.T).astype(np.float32)                                 # [D, K]
    cmhi, cmlo = _bf16_split(cm)
    c2 = (c * c).sum(axis=1).astype(np.float32)                          # [K]
    h, m, l = _bf16_split3(c2)
    c23 = np.ascontiguousarray(np.stack([h, m, l], axis=0))              # [3, K] bf16
    ones3 = np.ones((3, P), dtype=ml_dtypes.bfloat16)
    e2 = (e * e).sum(axis=1).astype(np.float32)                          # [N]
    eT = np.ascontiguousarray(e.T)                                       # [D, N]
    eThi, eTlo = _bf16_split(eT)

    in_maps = []
    for i in range(NCORES):
        sl = slice(i * NS, (i + 1) * NS)
        in_maps.append({
            "ethi": np.ascontiguousarray(eThi[:, sl]),
            "etlo": np.ascontiguousarray(eTlo[:, sl]),
            "cmhi": cmhi, "cmlo": cmlo, "ones3": ones3, "c23": c23,
            "e2t": np.ascontiguousarray(e2[sl].reshape(T, P).T),
        })

    res = run_bass_kernel_spmd(nc, in_maps, list(range(NCORES)))

    labels = np.empty(N, dtype=np.int32)
    loss_sum = 0.0
    for i in range(NCORES):
        r = res.results[i]
        sl = slice(i * NS, (i + 1) * NS)
        idx = r["idx8"].reshape(P, T, 8)[:, :, 0].T.reshape(-1)          # [NS]
        labels[sl] = idx.astype(np.int32)
        ymax = r["ymax8"].reshape(P, T, 8)[:, :, 0].T.reshape(-1).astype(np.float64)
        ssum = r["ssum"].T.reshape(-1).astype(np.float64)
        loss_sum += float(np.sum(np.log(ssum) - np.log(ymax)))

    loss = np.float32(loss_sum / N)
    return loss, labels
